# revision 28
# baseline (speedup 1.0000x reference)
"""Trainium2 Bass kernel for the ClusterLoss problem.

Loss = mean-entropy(softmax over K of [T, M, K] logits)            (L1)
       - mean-entropy(softmax over K of batch-mean logits [M, K])  (L2)

T=4096, M=64, K=256 hardcoded. Data-parallel over T across 8 cores.

The workload is memory-bound: 268 MB of logits must stream through the
cores once. The host casts the standard-normal fp32 logits to fp16
before upload (abs quantization error <= |x|*2^-11; the resulting loss
error is ~1e-5, far under tolerance), halving HBM traffic. Each core
reads a 16.8 MB shard laid out so every DMA moves contiguous 8 KiB
per-partition lines (partition p of tile t holds 16 consecutive
(t,m)-rows).

Per core (shard = [16, 128, 4096] fp16; 16 tiles of [128, 16, 256]):
  - DMA: 1 MiB contiguous mega-tiles into SBUF.
  - ACT: e = exp(x16) -> fp32, one batched instruction per tile.
  - DVE: one custom fused op per tile streams (x16, e) once and writes
         both running sums, multiplexed by element position:
           out[k] = (k at a block's last slot) ? cumsum(e) : cumsum(x*e)
  - ACT: per tile, one tiny copy extracts the two boundary columns
         (cum-S through elem 254, cum-Z through elem 255) per block.
  - PE:  0/1-pattern matmuls accumulate per-block sums over T into one
         8-bank PSUM tile (for L2's batch-mean logits). Row s of
         partition p has block id m = 16*(p%4) + s, so a [128,4]
         selector plus the sub-row free axis enumerates all 64 blocks.
Per-core outputs are tiny: the boundary stats [128, 16, 16, 2] and the
PSUM block sums [4, 8, 2, 256]. All entropy algebra (first differences,
the one missing x*e term per block - whose x is sliced straight from
the host-resident input - ln, division, reductions) runs on the host
in float64.

No max-subtraction in the softmax: inputs are standard-normal, |x| < ~6,
so exp(x) is comfortably inside fp32 range; H = ln(Z) - S/Z is
analytically identical to the reference's log_softmax entropy.
"""

import numpy as np

import concourse.bacc as bacc
import concourse.bass as bass
import concourse.tile as tile
from concourse import mybir
from concourse.bass_utils import run_bass_kernel_spmd

T, M, K = 4096, 64, 256
NCORES = 8
TSH = T // NCORES            # 512 t-rows per core
ROWS = TSH * M               # 32768 (t, m) rows per core
P = 128                      # SBUF partitions per tile
SUB = 16                     # consecutive rows per partition per tile
FREE = SUB * K               # 4096 fp16 elements = 8 KiB per partition
NT = ROWS // (P * SUB)       # 16 tiles of [128, 4096] per core
NG = 4                       # partition groups; m = 16*(p%4) + s
SPAIR = 2                    # sub-rows per PE matmul (moving free dim 512)
NJ = SUB // SPAIR            # 8 matmuls / PSUM banks

FP32 = mybir.dt.float32
FP16 = mybir.dt.float16


# --- custom DVE op: fused segmented-cumsum of (x*e, e) ---------------------- #
# out[p, s, k] = Idx >= 255+256*s ? cumsum(e)[p,s,k] : cumsum(x*e)[p,s,k]
# where both cumsums run over the whole [S*N] stream of one instruction
# (seeded per instruction, NOT per block; block values come from diffs).

def _register_fused_op():
    from concourse import dve_ops as _ops
    from concourse.dve_spec import (
        Spec, Src0, Src1, C0, C1, AluOp, scan, PageIdx, Idx, select, lower,
        _has_src1,
    )
    from concourse.dve_uop import DveOpSpec

    name = "SEGSCAN_SZ_ANT"
    for op in _ops.OPS:
        if op.name == name:
            return op

    pg = PageIdx(C0, C1)  # c0 + s*c1 ; call with s0=N-1, s1=N
    body = select(
        Idx >= pg,
        scan(AluOp.ADD, Src1),
        scan(AluOp.ADD, Src0 * Src1),
    )

    def _ref(in0, in1, c0, c1, c2):
        p = in0.shape[0]
        x = np.asarray(in0, np.float32).reshape(p, -1).astype(np.float64)
        e = np.asarray(in1, np.float32).reshape(p, -1).astype(np.float64)
        n = x.shape[1]
        start = float(np.asarray(c0).flat[0])
        step = float(np.asarray(c1).flat[0])
        idx = np.arange(n, dtype=np.float64)
        pgv = start + np.floor(idx / step) * step
        zc = np.cumsum(e, axis=1)
        sc = np.cumsum(x * e, axis=1)
        out = np.where(idx >= pgv, zc, sc).astype(np.float32)
        return out.reshape(in0.shape)

    spec = Spec(body=body, reference=_ref)
    row = _ops._CUSTOM_DVE_ROW_BASE + len(_ops.OPS)
    shas = {}
    for ver in ("v3", "v4"):
        t = DveOpSpec(
            name=name, opcode=row, uops=lower(spec, ver=ver),
            rd1_en=_has_src1(spec),
        )
        shas[ver] = t.sha(ver)
    op = _ops.DveOp(name, spec, subdim=True, uops_sha=shas)
    _ops.OPS.append(op)
    _ops.CUSTOM_DVE_SPECS[name] = spec
    _ops._SUB_OPCODE_FOR_NAME[name] = row
    return op


FUSED_SZ = _register_fused_op()


def _build_nc(use_pe=True, use_act=True, use_dve=True, repeat=1,
              xbufs=4, ebufs=4, do_op=None, do_extract=None,
              ext_eng="act", x_internal=False, no_dma=False,
              dma_split=1, et16=2, export_pos=(1, 3, 5)):
    """x_internal: declare x as Internal DRAM (garbage data) so timing
    runs don't ship the big input over the axon tunnel each call.
    no_dma: skip the x tile loads (compute-rate probes).
    dma_split: number of HWDGE queues to spread the x loads over.
    et16: 0 = fp32 e, in-place scan; 1 = fp16 e, separate fp32 scan out;
    2 = fp16 e, in-place fp16 scan (all-16-bit)."""
    from contextlib import nullcontext

    if do_op is None:
        do_op = use_dve
    if do_extract is None:
        do_extract = use_dve
    nc = bacc.Bacc("TRN2", target_bir_lowering=False, debug=False)

    x_kind = "Internal" if x_internal else "ExternalInput"
    x_d = nc.dram_tensor("x", [NT, P, FREE], FP16, kind=x_kind)
    w_d = nc.dram_tensor("wpat", [P, NG], FP16, kind="ExternalInput")
    stat_d = nc.dram_tensor("stat", [P, NT, SUB, 2], FP32,
                            kind="ExternalOutput")
    bsum_d = nc.dram_tensor("bsum", [NG, NJ, SPAIR, K], FP32,
                            kind="ExternalOutput")

    x = x_d.ap()

    with tile.TileContext(nc) as tc:
        with (
            tc.tile_pool(name="xin", bufs=xbufs) as xpool,
            tc.tile_pool(name="exp", bufs=ebufs) as epool,
            tc.tile_pool(name="stats", bufs=1) as stats,
            tc.tile_pool(name="small", bufs=1) as small,
            tc.tile_pool(name="psum", bufs=1, space="PSUM") as psum,
        ):
            wp = small.tile([P, NG], FP16)
            nc.sync.dma_start(out=wp, in_=w_d.ap())

            # statX[:, t, s, 0] = cum S through (row s of tile t, elem K-2)
            # statX[:, t, s, 1] = cum Z through (row s of tile t, elem K-1)
            # Two half-buffers so each export's dependency cone covers only
            # half the iteration's extracts (cross-iteration pipelining).
            NH = NT // 2
            statA = stats.tile([P, NH, SUB, 2], FP32)
            statB = stats.tile([P, NH, SUB, 2], FP32)
            bs_ps = psum.tile([NG, NJ, SPAIR, K], FP32)
            bsum_sb = small.tile([NG, NJ, SPAIR, K], FP32)

            if ext_eng == "act":
                _copy = lambda out, in_: nc.scalar.copy(out, in_)
            elif ext_eng == "gpsimd":
                _copy = lambda out, in_: nc.gpsimd.tensor_copy(out=out, in_=in_)
            else:
                _copy = lambda out, in_: nc.vector.tensor_copy(out=out, in_=in_)

            def _extract(t, et):
                if do_extract:
                    dst = statA[:, t] if t < NH else statB[:, t - NH]
                    _copy(dst[:, :, :], et[:, :, K - 2:K])

            if repeat > 1:
                # define-before-read for the first iteration's pipelined
                # export of the "previous" iteration's outputs.
                nc.vector.memset(statA, 0.0)
                nc.vector.memset(statB, 0.0)
                nc.vector.memset(bs_ps, 0.0)

            dma_engs = [nc.sync, nc.scalar, nc.vector][:max(dma_split, 1)]

            def body(export_prev):
                # Exports of the PREVIOUS iteration's stats/psum are issued
                # at the body head on the ACT hwdge queue: their deps are
                # long done, so they never stall the SP x-load stream, and
                # the boundary bubble disappears.
                pending = None
                for t in range(NT):
                    xt = xpool.tile([P, SUB, K], FP16, tag="xt")
                    if no_dma:
                        # seed-write so the tile framework sees the tile
                        # produced; compute timing probes only.
                        nc.sync.dma_start(out=xt[:, 0, :16], in_=x[t][:, :16])
                    else:
                        dma_engs[t % len(dma_engs)].dma_start(out=xt, in_=x[t])
                    et = epool.tile([P, SUB, K], FP16 if et16 else FP32,
                                    tag="et")
                    if use_act:
                        nc.scalar.activation(
                            out=et, in_=xt,
                            func=mybir.ActivationFunctionType.Exp,
                        )
                    if export_prev and t == export_pos[0]:
                        # deps: prev iter's extracts 0..NH-1, done mid-iter.
                        # SP queue: slots between x loads; SP has slack.
                        nc.sync.dma_start(
                            out=stat_d.ap()[:, :NH], in_=statA,
                        )
                    if export_prev and t == export_pos[1]:
                        if use_pe:
                            # ACT-queue head: hidden in ACT's drain lead.
                            nc.scalar.copy(out=bsum_sb, in_=bs_ps)
                        else:
                            nc.vector.memset(bsum_sb, 0.0)
                        nc.sync.dma_start(out=bsum_d.ap(), in_=bsum_sb)
                    if export_prev and t == export_pos[2]:
                        # deps: prev iter's last extract, done right at the
                        # iteration boundary - cleared before the SP queue
                        # reaches this position.
                        nc.sync.dma_start(
                            out=stat_d.ap()[:, NH:], in_=statB,
                        )
                    if et16 == 1:
                        ot = epool.tile([P, SUB, K], FP32, tag="ot")
                    else:
                        ot = et
                    if do_op:
                        # fused cumulative (x*e, e) with positional
                        # multiplex; writes over et in place (or to the
                        # fp32 ot buffer).
                        nc.vector._custom_dve(
                            FUSED_SZ, out=ot, in0=xt, in1=et,
                            s0=float(K - 1), s1=float(K),
                        )
                    if pending is not None:
                        _extract(*pending)
                    pending = (t, ot)
                    if use_pe:
                        for j in range(NJ):
                            nc.tensor.matmul(
                                bs_ps[:, j],
                                wp,
                                xt[:, j * SPAIR:(j + 1) * SPAIR, :],
                                start=(t == 0),
                                stop=(t == NT - 1),
                            )
                if pending is not None:
                    _extract(*pending)
                if not do_extract:
                    nc.vector.memset(statA, 1.0)
                    nc.vector.memset(statB, 1.0)

            if repeat > 1:
                with tc.For_i(0, repeat, 1):
                    body(export_prev=True)
            else:
                body(export_prev=False)

            # epilogue: export the final iteration's outputs.
            nc.sync.dma_start(out=stat_d.ap()[:, :NH], in_=statA)
            nc.sync.dma_start(out=stat_d.ap()[:, NH:], in_=statB)
            if use_pe:
                nc.scalar.copy(out=bsum_sb, in_=bs_ps)
            else:
                nc.vector.memset(bsum_sb, 0.0)
            nc.sync.dma_start(out=bsum_d.ap(), in_=bsum_sb)

    nc.compile()
    return nc


_NC_CACHE = []
BUILD_KW = {}  # overridable for A/B correctness experiments


def _get_nc():
    if not _NC_CACHE:
        _NC_CACHE.append(_build_nc(**BUILD_KW))
    return _NC_CACHE[0]


def _wpat():
    wp = np.zeros((P, NG), np.float16)
    wp[np.arange(P), np.arange(P) % NG] = 1.0
    return wp


def _input_map(shard):
    """Per-core input dict for a [NT, P, FREE] fp16 shard."""
    return {
        "x": np.ascontiguousarray(shard),
        "wpat": _wpat(),
    }


def kernel(block_feats, **kw):
    assert int(kw.get("M", M)) == M
    xf = np.asarray(block_feats)
    assert xf.shape == (T, M * K)
    xh = np.ascontiguousarray(xf).astype(np.float16)
    shards = xh.reshape(NCORES, NT, P, FREE)

    nc = _get_nc()
    in_maps = [_input_map(shards[i]) for i in range(NCORES)]
    res = run_bass_kernel_spmd(nc, in_maps, core_ids=list(range(NCORES))).results

    # ---- host tail: entropy algebra over the tiny boundary stats ----
    # xl[c, t, p, s] = last logit of each block, straight from the input.
    xl = shards.reshape(NCORES, NT, P, SUB, K)[..., K - 1].astype(np.float64)
    ent_total = 0.0
    for c in range(NCORES):
        st = res[c]["stat"].astype(np.float64)     # [P, NT, SUB, 2]
        sc254, zc = st[..., 0], st[..., 1]
        xlc = xl[c].transpose(1, 0, 2)             # [P, NT, SUB]
        sf = sc254 + xlc * np.exp(xlc)             # inclusive cum-S
        sp = np.diff(sf, axis=-1, prepend=0.0)     # per-block S
        zp = np.diff(zc, axis=-1, prepend=0.0)     # per-block Z
        ent_total += float((np.log(zp) - sp / zp).sum())
    L1 = ent_total / (T * M)

    bs = np.zeros((NG, NJ, SPAIR, K), np.float64)
    for r in res:
        bs += r["bsum"]
    # block id m = 16*g + (2*j + jj)  ->  [g, j, jj] row-major = m
    bm = bs.reshape(M, K) / T
    z = bm - bm.max(axis=-1, keepdims=True)
    e = np.exp(z)
    Z = e.sum(axis=-1, keepdims=True)
    logp = z - np.log(Z)
    H = -(np.exp(logp) * logp).sum(axis=-1)
    L2 = -H.mean()

    return np.asarray(L1 + L2, dtype=np.float32)


# revision 43
# speedup vs baseline: 1.0272x; 1.0272x over previous
"""Trainium2 Bass kernel for the ClusterLoss problem.

Loss = mean-entropy(softmax over K of [T, M, K] logits)            (L1)
       - mean-entropy(softmax over K of batch-mean logits [M, K])  (L2)

T=4096, M=64, K=256 hardcoded. Data-parallel over T across 8 cores.

The workload is memory-bound: 268 MB of logits must stream through the
cores once. The host casts the standard-normal fp32 logits to fp16
before upload (abs quantization error <= |x|*2^-11; the resulting loss
error is ~1e-5, far under tolerance), halving HBM traffic. Each core
reads a 16.8 MB shard laid out so every DMA moves contiguous 8 KiB
per-partition lines (partition p of tile t holds 16 consecutive
(t,m)-rows).

Per core (shard = [16, 128, 4096] fp16; 16 tiles of [128, 16, 256]):
  - DMA: 1 MiB contiguous mega-tiles into SBUF.
  - ACT: e = exp(x16) -> fp32, one batched instruction per tile.
  - DVE: one custom fused op per tile streams (x16, e) once and writes
         both running sums, multiplexed by element position:
           out[k] = (k at a block's last slot) ? cumsum(e) : cumsum(x*e)
  - ACT: per tile, one tiny copy extracts the two boundary columns
         (cum-S through elem 254, cum-Z through elem 255) per block.
  - PE:  0/1-pattern matmuls accumulate per-block sums over T into one
         8-bank PSUM tile (for L2's batch-mean logits). Row s of
         partition p has block id m = 16*(p%4) + s, so a [128,4]
         selector plus the sub-row free axis enumerates all 64 blocks.
Per-core outputs are tiny: the boundary stats [128, 16, 16, 2] and the
PSUM block sums [4, 8, 2, 256]. All entropy algebra (first differences,
the one missing x*e term per block - whose x is sliced straight from
the host-resident input - ln, division, reductions) runs on the host
in float64.

No max-subtraction in the softmax: inputs are standard-normal, |x| < ~6,
so exp(x) is comfortably inside fp32 range; H = ln(Z) - S/Z is
analytically identical to the reference's log_softmax entropy.
"""

import numpy as np

import concourse.bacc as bacc
import concourse.bass as bass
import concourse.tile as tile
from concourse import mybir
from concourse.bass_utils import run_bass_kernel_spmd

T, M, K = 4096, 64, 256
NCORES = 8
TSH = T // NCORES            # 512 t-rows per core
ROWS = TSH * M               # 32768 (t, m) rows per core
P = 128                      # SBUF partitions per tile
SUB = 16                     # consecutive rows per partition per tile
FREE = SUB * K               # 4096 fp16 elements = 8 KiB per partition
NT = ROWS // (P * SUB)       # 16 tiles of [128, 4096] per core
NG = 4                       # partition groups; m = 16*(p%4) + s
SPAIR = 2                    # sub-rows per PE matmul (moving free dim 512)
NJ = SUB // SPAIR            # 8 matmuls / PSUM banks
PAIR = 1                     # tiles fused per ACT/DVE instruction

FP32 = mybir.dt.float32
FP16 = mybir.dt.float16


# --- custom DVE op: fused segmented-cumsum of (x*e, e) ---------------------- #
# out[p, s, k] = Idx >= 255+256*s ? cumsum(e)[p,s,k] : cumsum(x*e)[p,s,k]
# where both cumsums run over the whole [S*N] stream of one instruction
# (seeded per instruction, NOT per block; block values come from diffs).

def _register_fused_op():
    from concourse import dve_ops as _ops
    from concourse.dve_spec import (
        Spec, Src0, Src1, C0, C1, AluOp, scan, PageIdx, Idx, select, lower,
        _has_src1,
    )
    from concourse.dve_uop import DveOpSpec

    name = "SEGSCAN_SZ_ANT"
    for op in _ops.OPS:
        if op.name == name:
            return op

    pg = PageIdx(C0, C1)  # c0 + s*c1 ; call with s0=N-1, s1=N
    body = select(
        Idx >= pg,
        scan(AluOp.ADD, Src1),
        scan(AluOp.ADD, Src0 * Src1),
    )

    def _ref(in0, in1, c0, c1, c2):
        p = in0.shape[0]
        x = np.asarray(in0, np.float32).reshape(p, -1).astype(np.float64)
        e = np.asarray(in1, np.float32).reshape(p, -1).astype(np.float64)
        n = x.shape[1]
        start = float(np.asarray(c0).flat[0])
        step = float(np.asarray(c1).flat[0])
        idx = np.arange(n, dtype=np.float64)
        pgv = start + np.floor(idx / step) * step
        zc = np.cumsum(e, axis=1)
        sc = np.cumsum(x * e, axis=1)
        out = np.where(idx >= pgv, zc, sc).astype(np.float32)
        return out.reshape(in0.shape)

    spec = Spec(body=body, reference=_ref)
    row = _ops._CUSTOM_DVE_ROW_BASE + len(_ops.OPS)
    shas = {}
    for ver in ("v3", "v4"):
        t = DveOpSpec(
            name=name, opcode=row, uops=lower(spec, ver=ver),
            rd1_en=_has_src1(spec),
        )
        shas[ver] = t.sha(ver)
    op = _ops.DveOp(name, spec, subdim=True, uops_sha=shas)
    _ops.OPS.append(op)
    _ops.CUSTOM_DVE_SPECS[name] = spec
    _ops._SUB_OPCODE_FOR_NAME[name] = row
    return op


FUSED_SZ = _register_fused_op()


def _build_nc(use_pe=True, use_act=True, use_dve=True, repeat=1,
              xbufs=4, ebufs=4, do_op=None, do_extract=None,
              ext_eng="act", x_internal=False, no_dma=False,
              dma_split=1, et16=2, export_pos=(1, 3, 5), pair=PAIR):
    """x_internal: declare x as Internal DRAM (garbage data) so timing
    runs don't ship the big input over the axon tunnel each call.
    no_dma: skip the x tile loads (compute-rate probes).
    dma_split: number of HWDGE queues to spread the x loads over.
    et16: 0 = fp32 e, in-place scan; 1 = fp16 e, separate fp32 scan out;
    2 = fp16 e, in-place fp16 scan (all-16-bit)."""
    from contextlib import nullcontext

    if do_op is None:
        do_op = use_dve
    if do_extract is None:
        do_extract = use_dve
    nc = bacc.Bacc("TRN2", target_bir_lowering=False, debug=False)

    x_kind = "Internal" if x_internal else "ExternalInput"
    x_d = nc.dram_tensor("x", [NT, P, FREE], FP16, kind=x_kind)
    w_d = nc.dram_tensor("wpat", [P, NG], FP16, kind="ExternalInput")
    stat_d = nc.dram_tensor("stat", [P, NT, SUB, 2], FP32,
                            kind="ExternalOutput")
    bsum_d = nc.dram_tensor("bsum", [NG, NJ, SPAIR, K], FP32,
                            kind="ExternalOutput")

    x = x_d.ap()

    with tile.TileContext(nc) as tc:
        with (
            tc.tile_pool(name="xin", bufs=xbufs) as xpool,
            tc.tile_pool(name="exp", bufs=ebufs) as epool,
            tc.tile_pool(name="stats", bufs=1) as stats,
            tc.tile_pool(name="small", bufs=1) as small,
            tc.tile_pool(name="psum", bufs=1, space="PSUM") as psum,
        ):
            wp = small.tile([P, NG], FP16)
            nc.sync.dma_start(out=wp, in_=w_d.ap())

            # statX[:, t, s, 0] = cum S through (row s of tile t, elem K-2)
            # statX[:, t, s, 1] = cum Z through (row s of tile t, elem K-1)
            # Two half-buffers so each export's dependency cone covers only
            # half the iteration's extracts (cross-iteration pipelining).
            NH = NT // 2
            statA = stats.tile([P, NH * SUB, 2], FP32)
            statB = stats.tile([P, NH * SUB, 2], FP32)
            bs_ps = psum.tile([NG, NJ, SPAIR, K], FP32)
            bsum_sb = small.tile([NG, NJ, SPAIR, K], FP32)

            if ext_eng == "act":
                _copy = lambda out, in_: nc.scalar.copy(out, in_)
            elif ext_eng == "gpsimd":
                _copy = lambda out, in_: nc.gpsimd.tensor_copy(out=out, in_=in_)
            else:
                _copy = lambda out, in_: nc.vector.tensor_copy(out=out, in_=in_)

            def _extract(t, et):
                # et: [P, pair*SUB, K]; covers tiles t..t+pair-1 (never
                # straddles the A/B half boundary: pair divides NH).
                if do_extract:
                    if t < NH:
                        dst = statA[:, t * SUB:(t + pair) * SUB]
                    else:
                        dst = statB[:, (t - NH) * SUB:(t - NH + pair) * SUB]
                    _copy(dst, et[:, :, K - 2:K])

            if repeat > 1:
                # define-before-read for the first iteration's pipelined
                # export of the "previous" iteration's outputs.
                nc.vector.memset(statA, 0.0)
                nc.vector.memset(statB, 0.0)
                nc.vector.memset(bs_ps, 0.0)

            dma_engs = [nc.sync, nc.scalar, nc.vector][:max(dma_split, 1)]

            def body(export_prev):
                # Exports of the PREVIOUS iteration's stats/psum are issued
                # early in the body between x loads: their deps are long
                # done, so they never stall the critical queues, and the
                # boundary bubble disappears.
                pending = None
                for u in range(NT // pair):
                    t = u * pair
                    xt = xpool.tile([P, pair * SUB, K], FP16, tag="xt")
                    if no_dma:
                        # seed-write so the tile framework sees the tile
                        # produced; compute timing probes only.
                        nc.sync.dma_start(out=xt[:, 0, :16],
                                          in_=x[t][:, :16])
                    else:
                        for i in range(pair):
                            dma_engs[u % len(dma_engs)].dma_start(
                                out=xt[:, i * SUB:(i + 1) * SUB], in_=x[t + i])
                    et = epool.tile([P, pair * SUB, K],
                                    FP16 if et16 else FP32, tag="et")
                    if use_act:
                        nc.scalar.activation(
                            out=et, in_=xt,
                            func=mybir.ActivationFunctionType.Exp,
                        )
                    if export_prev and u == export_pos[0]:
                        # deps: prev iter's extracts 0..NH-1, done mid-iter.
                        # SP queue: slots between x loads; SP has slack.
                        nc.sync.dma_start(
                            out=stat_d.ap()[:, :NH], in_=statA,
                        )
                    if export_prev and u == export_pos[1]:
                        if use_pe:
                            # ACT-queue head: hidden in ACT's drain lead.
                            nc.scalar.copy(out=bsum_sb, in_=bs_ps)
                        else:
                            nc.vector.memset(bsum_sb, 0.0)
                        nc.sync.dma_start(out=bsum_d.ap(), in_=bsum_sb)
                    if export_prev and u == export_pos[2]:
                        # deps: prev iter's last extract, done right at the
                        # iteration boundary - cleared before the SP queue
                        # reaches this position.
                        nc.sync.dma_start(
                            out=stat_d.ap()[:, NH:], in_=statB,
                        )
                    if et16 == 1:
                        ot = epool.tile([P, pair * SUB, K], FP32, tag="ot")
                    else:
                        ot = et
                    if do_op:
                        # fused cumulative (x*e, e) with positional
                        # multiplex; writes over et in place (or to the
                        # fp32 ot buffer).
                        nc.vector._custom_dve(
                            FUSED_SZ, out=ot, in0=xt, in1=et,
                            s0=float(K - 1), s1=float(K),
                        )
                    if pending is not None:
                        _extract(*pending)
                    pending = (t, ot)
                    if use_pe:
                        for i in range(pair):
                            for j in range(NJ):
                                sl = i * SUB + j * SPAIR
                                nc.tensor.matmul(
                                    bs_ps[:, j],
                                    wp,
                                    xt[:, sl:sl + SPAIR, :],
                                    start=(t + i == 0),
                                    stop=(t + i == NT - 1),
                                )
                if pending is not None:
                    _extract(*pending)
                if not do_extract:
                    nc.vector.memset(statA, 1.0)
                    nc.vector.memset(statB, 1.0)

            if repeat > 1:
                with tc.For_i(0, repeat, 1):
                    body(export_prev=True)
            else:
                body(export_prev=False)

            # epilogue: export the final iteration's outputs.
            nc.sync.dma_start(out=stat_d.ap()[:, :NH], in_=statA)
            nc.sync.dma_start(out=stat_d.ap()[:, NH:], in_=statB)
            if use_pe:
                nc.scalar.copy(out=bsum_sb, in_=bs_ps)
            else:
                nc.vector.memset(bsum_sb, 0.0)
            nc.sync.dma_start(out=bsum_d.ap(), in_=bsum_sb)

    nc.compile()
    return nc


_NC_CACHE = []
BUILD_KW = {}  # overridable for A/B correctness experiments


def _get_nc():
    if not _NC_CACHE:
        _NC_CACHE.append(_build_nc(**BUILD_KW))
    return _NC_CACHE[0]


def _wpat():
    wp = np.zeros((P, NG), np.float16)
    wp[np.arange(P), np.arange(P) % NG] = 1.0
    return wp


def _input_map(shard):
    """Per-core input dict for a [NT, P, FREE] fp16 shard."""
    return {
        "x": np.ascontiguousarray(shard),
        "wpat": _wpat(),
    }


def kernel(block_feats, **kw):
    assert int(kw.get("M", M)) == M
    xf = np.asarray(block_feats)
    assert xf.shape == (T, M * K)
    xh = np.ascontiguousarray(xf).astype(np.float16)
    shards = xh.reshape(NCORES, NT, P, FREE)

    nc = _get_nc()
    in_maps = [_input_map(shards[i]) for i in range(NCORES)]
    res = run_bass_kernel_spmd(nc, in_maps, core_ids=list(range(NCORES))).results

    # ---- host tail: entropy algebra over the tiny boundary stats ----
    # xl[c, t, p, s] = last logit of each block, straight from the input.
    # The device cumsum restarts per DVE instruction = per `pair` tiles,
    # so first-differences run over [pair*SUB]-block groups.
    pr = BUILD_KW.get("pair", PAIR)
    xl = shards.reshape(NCORES, NT, P, SUB, K)[..., K - 1].astype(np.float64)
    ent_total = 0.0
    for c in range(NCORES):
        st = res[c]["stat"].astype(np.float64)     # [P, NT, SUB, 2]
        sc254 = st[..., 0].reshape(P, NT // pr, pr * SUB)
        zc = st[..., 1].reshape(P, NT // pr, pr * SUB)
        xlc = xl[c].transpose(1, 0, 2).reshape(P, NT // pr, pr * SUB)
        sf = sc254 + xlc * np.exp(xlc)             # inclusive cum-S
        sp = np.diff(sf, axis=-1, prepend=0.0)     # per-block S
        zp = np.diff(zc, axis=-1, prepend=0.0)     # per-block Z
        ent_total += float((np.log(zp) - sp / zp).sum())
    L1 = ent_total / (T * M)

    bs = np.zeros((NG, NJ, SPAIR, K), np.float64)
    for r in res:
        bs += r["bsum"]
    # block id m = 16*g + (2*j + jj)  ->  [g, j, jj] row-major = m
    bm = bs.reshape(M, K) / T
    z = bm - bm.max(axis=-1, keepdims=True)
    e = np.exp(z)
    Z = e.sum(axis=-1, keepdims=True)
    logp = z - np.log(Z)
    H = -(np.exp(logp) * logp).sum(axis=-1)
    L2 = -H.mean()

    return np.asarray(L1 + L2, dtype=np.float32)
